# revision 31
# baseline (speedup 1.0000x reference)
"""Trainium2 Bass kernel for nn_AdaConvNeXt (moe_routing) — v3.

Data-parallel over batch (16 images/core). Baseline pipeline structure
(2-image stat blocks, mask-based routing, DRAM stat-row broadcast) with
the compute recast in fp8 DoubleRow on TensorE:
  - depthwise 7x7 conv: all 49 taps on TensorE as 24 fp8-DR pair passes
    + 1 single pass per (group, half-bank), on a host-padded 34x36 fp8
    input (one contiguous DMA descriptor per partition).
  - FFN + fast path on all tokens with 0/1 masks (idx2-wins preserved),
    matmuls in fp8 DoubleRow: w1 = DR pair + single per (fg, lam),
    w2 = 6 DR passes, fast path = DR + single. LN affine/gamma/biases
    host-folded with dynamic power-of-2 fp8 scales.
  - LayerNorm stats via ones-vector matmuls; batched stat math per block;
    residual added in f32.
"""

import os
import numpy as np
import ml_dtypes

import concourse.bass as bass
import concourse.bacc as bacc
import concourse.mybir as mybir
import concourse.tile as tile
from concourse.bass_utils import run_bass_kernel_spmd

VP = mybir._bass_rust.VecI64Pair
BF16 = mybir.dt.bfloat16
FP8 = mybir.dt.float8e4
F32 = mybir.dt.float32
ADD = mybir.AluOpType.add
MULT = mybir.AluOpType.mult
AF = mybir.ActivationFunctionType
DRM = mybir.MatmulPerfMode.DoubleRow

N_CORES = 8
B, C, H, W = 128, 384, 28, 28
N = H * W          # 784
BL = B // N_CORES  # 16 images per core
NG = C // 128      # 3 channel groups
FG = (4 * C) // 128  # 12 ffn groups
HALF = N // 2      # 392
EPS = 1e-6
STAT_BLK = 2
HP, WP = 34, 36
PPITCH = NG * HP * WP


def _tap_pairs():
    pairs = []
    for dy in range(-3, 4):
        pairs.append(((dy, -3), (dy, -1)))
        pairs.append(((dy, 1), (dy, 3)))
        pairs.append(((dy, -2), (dy, 0)))
    pairs.append(((1, 2), (2, 2)))
    return pairs, (3, 2)


PAIRS, SINGLE = _tap_pairs()
DVE_TAPS = [(3, 2), (-3, 2), (-2, 2), (-1, 2), (0, 2)]  # cvec 21+3*tix+g
NPASS = len(PAIRS)


def _off(dy, dx):
    return (3 + dy) * WP + (3 + dx)


def cap(ap, aplist):
    c = ap.copy()
    c.ap = VP(aplist)
    return c


def build_bass(BL_, SD, S1, S2f, S2q):
    nc = bacc.Bacc(None, target_bir_lowering=False, debug=False)

    xpad_d = nc.declare_dram_parameter("xpad", [BL_, C, HP, WP], FP8, isOutput=False)
    masks_d = nc.declare_dram_parameter("masks", [2, BL_, H, W], BF16, isOutput=False)
    convdr_d = nc.declare_dram_parameter("convdr", [128, NG, NPASS, 2, 128], FP8, isOutput=False)
    convsg_d = nc.declare_dram_parameter("convsg", [128, NG, 128], FP8, isOutput=False)
    w1dr_d = nc.declare_dram_parameter("w1dr", [128, FG, 2, 128], FP8, isOutput=False)
    w1sg_d = nc.declare_dram_parameter("w1sg", [128, FG, 128], FP8, isOutput=False)
    w2fdr_d = nc.declare_dram_parameter("w2fdr", [128, NG, 6, 2, 128], FP8, isOutput=False)
    w2qdr_d = nc.declare_dram_parameter("w2qdr", [128, NG, 2, 128], FP8, isOutput=False)
    w2qsg_d = nc.declare_dram_parameter("w2qsg", [128, NG, 128], FP8, isOutput=False)
    # cvec cols: 0..2 dwb*SD, 3..14 c1, 15..17 c1out, 18..20 c2
    cvec_d = nc.declare_dram_parameter("cvec", [128, 36], F32, isOutput=False)
    out_d = nc.declare_dram_parameter("out", [BL_, C, H, W], BF16, isOutput=True)

    from contextlib import ExitStack
    with ExitStack() as es:
        tc = es.enter_context(tile.TileContext(nc))
        pool = lambda name, bufs, **kw: es.enter_context(
            tc.tile_pool(name=name, bufs=bufs, **kw))
        cpool = pool("consts", 1)
        xin_pool = pool("xin", 4)
        y_pool = pool("ybuf", STAT_BLK + 2)
        ysq_pool = pool("ysq", 3)
        z_pool = pool("zbuf", 2)
        g_pool = pool("gbuf", 2)
        mb_pool = pool("maskb", 3)
        tmp_pool = pool("tmp", 4)
        tsm_pool = pool("tsm", 3)
        rows_pool = pool("rows", 1)
        o_pool = pool("obuf", 3)
        dram_pool = pool("dscratch", 4, space=bass.MemorySpace.DRAM)
        py_pool = pool("py", 2, space=bass.MemorySpace.PSUM)
        ph_pool = pool("ph", 3, space=bass.MemorySpace.PSUM)
        paux_pool = pool("paux", 1, space=bass.MemorySpace.PSUM)
        ppq_pool = pool("ppq", 2, space=bass.MemorySpace.PSUM)

        # ---- constants into SBUF ----
        convdr_sb = cpool.tile([128, NG, NPASS, 2, 128], FP8)
        for _g in range(NG):
            nc.sync.dma_start(convdr_sb[:, _g], convdr_d[:, _g])
        convsg_sb = cpool.tile([128, NG, 128], FP8)
        nc.sync.dma_start(convsg_sb[:], convsg_d[:])
        w1dr_sb = cpool.tile([128, FG, 2, 128], FP8)
        nc.scalar.dma_start(w1dr_sb[:], w1dr_d[:])
        w1sg_sb = cpool.tile([128, FG, 128], FP8)
        nc.scalar.dma_start(w1sg_sb[:], w1sg_d[:])
        w2fdr_sb = cpool.tile([128, NG, 6, 2, 128], FP8)
        nc.scalar.dma_start(w2fdr_sb[:], w2fdr_d[:])
        w2qdr_sb = cpool.tile([128, NG, 2, 128], FP8)
        nc.scalar.dma_start(w2qdr_sb[:], w2qdr_d[:])
        w2qsg_sb = cpool.tile([128, NG, 128], FP8)
        nc.scalar.dma_start(w2qsg_sb[:], w2qsg_d[:])
        cvec_sb = cpool.tile([128, 36], F32)
        nc.scalar.dma_start(cvec_sb[:], cvec_d[:])

        ones_col = cpool.tile([128, 1], BF16)
        nc.vector.memset(ones_col[:], 1.0)
        eps_col = cpool.tile([33, 1], F32)
        nc.vector.memset(eps_col[:], float(SD) * float(SD) * EPS)

        n_blocks = (BL_ + STAT_BLK - 1) // STAT_BLK

        def phase1(blk):
            imgs = list(range(blk * STAT_BLK, min((blk + 1) * STAT_BLK, BL_)))
            nb = len(imgs)

            srow = rows_pool.tile([33, N], F32)  # sum/C
            qrow = rows_pool.tile([33, N], F32)  # sumsq/C

            y_tiles = {}
            for ii, img in enumerate(imgs):
                # ---- load host-padded fp8 input ----
                xpq = xin_pool.tile([128, NG, HP, WP], FP8)
                nc.sync.dma_start(
                    out=xpq[:], in_=xpad_d[img].rearrange("(g c) h w -> c g h w", g=NG))

                # ---- depthwise conv: fp8 DR pairs on PE ----
                y_bf = y_pool.tile([128, NG, H, W], BF16)
                y_tiles[img] = y_bf
                ysq = ysq_pool.tile([128, NG, H, W], BF16)
                for g in range(NG):
                    # 3 taps on DVE: short ts + 2*stt chain (hides under PE)
                    tsg = tmp_pool.tile([128, H, W], BF16, tag="tsg")
                    dy, dx = DVE_TAPS[0]
                    nc.vector.tensor_scalar(
                        out=tsg[:], in0=xpq[:, g, 3 + dy:31 + dy, 3 + dx:31 + dx],
                        scalar1=cvec_sb[:, 21 + g:22 + g], scalar2=None, op0=MULT)
                    for tix in (1, 2, 3, 4):
                        dy, dx = DVE_TAPS[tix]
                        nc.vector.scalar_tensor_tensor(
                            out=tsg[:],
                            in0=xpq[:, g, 3 + dy:31 + dy, 3 + dx:31 + dx],
                            scalar=cvec_sb[:, 21 + 3 * tix + g:22 + 3 * tix + g],
                            in1=tsg[:], op0=MULT, op1=ADD)
                    for h in range(2):
                        py = py_pool.tile([128, 14, W], F32, tag="py")
                        base = g * HP * WP + h * 14 * WP
                        for k, (ta, tb) in enumerate(PAIRS):
                            oa, ob_ = _off(*ta), _off(*tb)
                            rhs = cap(xpq[:, g, 0:14, 0:W],
                                      [[PPITCH, 128], [ob_ - oa, 2], [WP, 14], [1, W]])
                            rhs.offset = xpq[:].offset + base + oa
                            nc.tensor.matmul(
                                py[:], convdr_sb[:, g, k], rhs,
                                start=(k == 0), stop=(k == NPASS - 1),
                                perf_mode=DRM, skip_group_check=True)
                        # y_s = (psum + SD*dw_b) + single-tap (bf16)
                        nc.vector.scalar_tensor_tensor(
                            out=y_bf[:, g, h * 14:h * 14 + 14, :], in0=py[:],
                            scalar=cvec_sb[:, g:g + 1],
                            in1=tsg[:, h * 14:h * 14 + 14, :],
                            op0=ADD, op1=ADD)
                    # ysq = (y_s/SD)^2 = y_true^2
                    nc.scalar.activation(ysq[:, g], y_bf[:, g], AF.Square,
                                         scale=1.0 / SD)

                # ---- LN stats: per-token sum and sumsq via ones-matmuls ----
                for lam in range(2):
                    rs = slice(14 * lam, 14 * lam + 14)
                    pst = paux_pool.tile([33, HALF], F32, tag='aux')
                    for g in range(NG):
                        nc.tensor.matmul(
                            pst[0:1, :], ones_col[:], y_bf[:, g, rs, :],
                            start=(g == 0), stop=(g == NG - 1),
                            skip_group_check=True)
                    for g in range(NG):
                        nc.tensor.matmul(
                            pst[32:33, :], ones_col[:], ysq[:, g, rs, :],
                            start=(g == 0), stop=(g == NG - 1),
                            tile_position=(0, 32),
                            skip_group_check=True)
                    cs = slice(HALF * lam, HALF * lam + HALF)
                    ps = 32 * ii
                    nc.scalar.activation(
                        srow[ps:ps + 1, cs], pst[0:1, :], AF.Copy, scale=1.0 / (SD * C))
                    nc.scalar.activation(
                        qrow[ps:ps + 1, cs], pst[32:33, :], AF.Copy, scale=1.0 / C)

            # ---- batched stats math: istd/SD and -mu*istd ----
            np_ = 32 * (nb - 1) + 1
            musq = rows_pool.tile([33, N], F32, tag="rowsw1")
            nc.vector.tensor_tensor(out=musq[:np_], in0=srow[:np_], in1=srow[:np_], op=MULT)
            veps = rows_pool.tile([33, N], F32, tag="rowsw2")
            nc.vector.scalar_tensor_tensor(
                out=veps[:np_], in0=musq[:np_], scalar=-1.0, in1=qrow[:np_],
                op0=MULT, op1=ADD)
            sd = rows_pool.tile([33, N], F32, tag="rowsw1")
            nc.scalar.activation(sd[:np_], veps[:np_], AF.Sqrt, bias=eps_col[:np_],
                                 scale=float(SD) * float(SD))
            istd_r = rows_pool.tile([33, N], F32)
            with nc.allow_low_precision(reason="branch output is gamma-scaled"):
                nc.vector.reciprocal_approx_fast(out=istd_r[:np_], in_=sd[:np_])
            nmi_r = rows_pool.tile([33, N], F32, tag="rowsw2")
            nc.vector.scalar_tensor_tensor(
                out=nmi_r[:np_], in0=srow[:np_], scalar=-float(SD), in1=istd_r[:np_],
                op0=MULT, op1=MULT)

            # stage the per-image stat rows in DRAM for partition-broadcast
            stat_dr = {}
            for ii, img in enumerate(imgs):
                ps = 32 * ii
                sc = dram_pool.tile([2, N], F32, tag="sc", name=f"sc{blk}_{ii}")
                nc.sync.dma_start(out=sc[0:1, :], in_=istd_r[ps:ps + 1, :])
                nc.sync.dma_start(out=sc[1:2, :], in_=nmi_r[ps:ps + 1, :])
                stat_dr[img] = sc
            return imgs, y_tiles, stat_dr

        def phase2(state):
            imgs, y_tiles, stat_dr = state
            # ---- phase 2: z, FFN, merge, store ----
            for ii, img in enumerate(imgs):
                y_bf = y_tiles[img]
                sc = stat_dr[img]
                m1b = mb_pool.tile([128, H, W], BF16, tag="m1b")
                nc.sync.dma_start(
                    out=m1b[:], in_=masks_d[0:1, img].partition_broadcast(128))
                m2b = mb_pool.tile([128, H, W], BF16, tag="m2b")
                nc.sync.dma_start(
                    out=m2b[:], in_=masks_d[1:2, img].partition_broadcast(128))
                istd_b = mb_pool.tile([128, H, W], BF16, tag="istdb")
                nc.gpsimd.dma_start(
                    out=istd_b[:], in_=sc[0:1, :].partition_broadcast(128))
                nmi_b = mb_pool.tile([128, H, W], BF16, tag="nmib")
                nc.gpsimd.dma_start(
                    out=nmi_b[:], in_=sc[1:2, :].partition_broadcast(128))

                # z = y_s * (istd/SD) + (-mu*istd)  -> fp8
                z_q = z_pool.tile([128, NG, H, W], FP8)
                for g in range(NG):
                    tz = tmp_pool.tile([128, H, W], BF16, tag="tz")
                    nc.vector.tensor_tensor(
                        out=tz[:], in0=y_bf[:, g], in1=istd_b[:], op=MULT)
                    nc.vector.tensor_tensor(
                        out=z_q[:, g], in0=tz[:], in1=nmi_b[:], op=ADD)

                # FFN: h = W1^T z (DR+single), gelu -> g_q; p = W2f^T g; q = W2q^T z
                g_q = g_pool.tile([128, FG, H, W], FP8)
                for fg in range(FG):
                    for lam in range(2):
                        rs = slice(14 * lam, 14 * lam + 14)
                        ph = ph_pool.tile([128, 14, W], F32, tag="ph")
                        rhs = cap(z_q[:], [[NG * N, 128], [N, 2], [W, 14], [1, W]])
                        rhs.offset = z_q[:].offset + lam * HALF
                        nc.tensor.matmul(ph[:], w1dr_sb[:, fg], rhs,
                                         start=True, stop=False,
                                         perf_mode=DRM, skip_group_check=True)
                        nc.tensor.matmul(ph[:], w1sg_sb[:, fg], z_q[:, 2, rs, :],
                                         start=False, stop=True, skip_group_check=True)
                        nc.scalar.activation(
                            g_q[:, fg, rs, :], ph[:], AF.Gelu,
                            bias=cvec_sb[:, 3 + fg:4 + fg], scale=1.0 / S1)
                for og in range(NG):
                    rs2 = [slice(0, 14), slice(14, 28)]
                    pp = [ppq_pool.tile([128, 14, W], F32, tag="pq", name=f"pp{_l}")
                          for _l in range(2)]
                    for lam in range(2):
                        for j in range(6):
                            rhs = cap(g_q[:], [[FG * N, 128], [N, 2], [W, 14], [1, W]])
                            rhs.offset = g_q[:].offset + 2 * j * N + lam * HALF
                            nc.tensor.matmul(pp[lam][:], w2fdr_sb[:, og, j], rhs,
                                             start=(j == 0), stop=(j == 5),
                                             perf_mode=DRM, skip_group_check=True)
                    t1p = [tsm_pool.tile([128, 14, W], BF16, tag="t1p", name=f"t1p{_l}")
                           for _l in range(2)]
                    for lam in range(2):
                        nc.scalar.activation(
                            t1p[lam][:], pp[lam][:], AF.Identity,
                            bias=cvec_sb[:, 15 + og:16 + og], scale=1.0 / S2f)
                    pq = [ppq_pool.tile([128, 14, W], F32, tag="pq", name=f"pq{_l}")
                          for _l in range(2)]
                    for lam in range(2):
                        rhs = cap(z_q[:], [[NG * N, 128], [N, 2], [W, 14], [1, W]])
                        rhs.offset = z_q[:].offset + lam * HALF
                        nc.tensor.matmul(pq[lam][:], w2qdr_sb[:, og], rhs,
                                         start=True, stop=False,
                                         perf_mode=DRM, skip_group_check=True)
                        nc.tensor.matmul(pq[lam][:], w2qsg_sb[:, og],
                                         z_q[:, 2, rs2[lam], :],
                                         start=False, stop=True, skip_group_check=True)
                    t2p = [tsm_pool.tile([128, 14, W], BF16, tag="t2p", name=f"t2p{_l}")
                           for _l in range(2)]
                    for lam in range(2):
                        nc.scalar.activation(
                            t2p[lam][:], pq[lam][:], AF.Identity,
                            bias=cvec_sb[:, 18 + og:19 + og], scale=1.0 / S2q)
                    for lam in range(2):
                        rs = rs2[lam]
                        t1 = tsm_pool.tile([128, 14, W], BF16, tag="t1")
                        nc.vector.tensor_tensor(
                            out=t1[:], in0=t1p[lam][:], in1=m1b[:, rs, :], op=MULT)
                        t2 = tsm_pool.tile([128, 14, W], BF16, tag="t2")
                        nc.vector.tensor_tensor(
                            out=t2[:], in0=t2p[lam][:], in1=m2b[:, rs, :], op=MULT)
                        s12 = o_pool.tile([128, 14, W], BF16, tag="s12")
                        nc.gpsimd.tensor_tensor(
                            out=s12[:], in0=t1[:], in1=t2[:], op=ADD)
                        nc.sync.dma_start(
                            out=out_d[img, og * 128:(og + 1) * 128, rs, :], in_=s12[:])

        pending = {}
        for step in range(n_blocks + 1):
            if step < n_blocks:
                pending[step] = phase1(step)
            if step >= 1:
                phase2(pending.pop(step - 1))
    nc.compile()
    return nc


# ---------------------------------------------------------------------------
# host side
# ---------------------------------------------------------------------------

def _pow2_scale(mat, target=64.0):
    m = float(np.abs(mat).max())
    if m == 0.0:
        return 1.0
    return float(2.0 ** np.floor(np.log2(target / m)))


def _fold_host(inputs):
    f32 = np.float32
    fp8 = ml_dtypes.float8_e4m3fn
    dw_w = np.asarray(inputs["dw_w"], f32)
    dw_b = np.asarray(inputs["dw_b"], f32)
    norm_w = np.asarray(inputs["norm_w"], f32)
    norm_b = np.asarray(inputs["norm_b"], f32)
    w1 = np.asarray(inputs["w1"], f32)
    b1 = np.asarray(inputs["b1"], f32)
    w2 = np.asarray(inputs["w2"], f32)
    b2 = np.asarray(inputs["b2"], f32)
    gamma = np.asarray(inputs["gamma"], f32)
    fp_norm_w = np.asarray(inputs["fp_norm_w"], f32)
    fp_norm_b = np.asarray(inputs["fp_norm_b"], f32)
    fp_w = np.asarray(inputs["fp_w"], f32)
    fp_b = np.asarray(inputs["fp_b"], f32)
    fp_gamma = np.asarray(inputs["fp_gamma"], f32)

    W1 = norm_w[:, None] * w1
    c1 = norm_b @ w1 + b1
    W2f = w2 * gamma[None, :]
    c1out = b2 * gamma
    W2q = (fp_norm_w[:, None] * fp_w) * fp_gamma[None, :]
    c2 = (fp_norm_b @ fp_w + fp_b) * fp_gamma

    SD = _pow2_scale(dw_w, 4.0)
    S1 = _pow2_scale(W1, 64.0)
    S2f = _pow2_scale(W2f, 64.0)
    S2q = _pow2_scale(W2q, 64.0)

    ar = np.arange(128)
    convdr = np.zeros((128, NG, NPASS, 2, 128), f32)
    convsg = np.zeros((128, NG, 128), f32)
    for g in range(NG):
        ch = slice(g * 128, (g + 1) * 128)
        for k, (ta, tb) in enumerate(PAIRS):
            convdr[ar, g, k, 0, ar] = dw_w[ch, 0, ta[0] + 3, ta[1] + 3] * SD
            convdr[ar, g, k, 1, ar] = dw_w[ch, 0, tb[0] + 3, tb[1] + 3] * SD
        convsg[ar, g, ar] = dw_w[ch, 0, SINGLE[0] + 3, SINGLE[1] + 3] * SD

    w1dr = np.zeros((128, FG, 2, 128), f32)
    w1sg = np.zeros((128, FG, 128), f32)
    for fg in range(FG):
        fs = slice(fg * 128, (fg + 1) * 128)
        w1dr[:, fg, 0] = W1[0:128, fs] * S1
        w1dr[:, fg, 1] = W1[128:256, fs] * S1
        w1sg[:, fg] = W1[256:384, fs] * S1
    w2fdr = np.zeros((128, NG, 6, 2, 128), f32)
    for og in range(NG):
        os_ = slice(og * 128, (og + 1) * 128)
        for j in range(6):
            w2fdr[:, og, j, 0] = W2f[(2 * j) * 128:(2 * j + 1) * 128, os_] * S2f
            w2fdr[:, og, j, 1] = W2f[(2 * j + 1) * 128:(2 * j + 2) * 128, os_] * S2f
    w2qdr = np.zeros((128, NG, 2, 128), f32)
    w2qsg = np.zeros((128, NG, 128), f32)
    for og in range(NG):
        os_ = slice(og * 128, (og + 1) * 128)
        w2qdr[:, og, 0] = W2q[0:128, os_] * S2q
        w2qdr[:, og, 1] = W2q[128:256, os_] * S2q
        w2qsg[:, og] = W2q[256:384, os_] * S2q

    cvec = np.zeros((128, 36), f32)
    for g in range(NG):
        cvec[:, g] = dw_b[g * 128:(g + 1) * 128] * SD
        for tix, (tdy, tdx) in enumerate(DVE_TAPS):
            cvec[:, 21 + 3 * tix + g] = dw_w[g * 128:(g + 1) * 128, 0,
                                             tdy + 3, tdx + 3] * SD
    for fg in range(FG):
        cvec[:, 3 + fg] = c1[fg * 128:(fg + 1) * 128]
    for og in range(NG):
        cvec[:, 15 + og] = c1out[og * 128:(og + 1) * 128]
        cvec[:, 18 + og] = c2[og * 128:(og + 1) * 128]

    return dict(
        convdr=convdr.astype(fp8), convsg=convsg.astype(fp8),
        w1dr=w1dr.astype(fp8), w1sg=w1sg.astype(fp8),
        w2fdr=w2fdr.astype(fp8), w2qdr=w2qdr.astype(fp8),
        w2qsg=w2qsg.astype(fp8), cvec=cvec,
    ), SD, S1, S2f, S2q


def _masks_host(idx1, idx2, Bn):
    m2 = np.zeros((Bn, N), np.float32)
    np.put_along_axis(m2, np.asarray(idx2, np.int64), 1.0, axis=1)
    m1 = np.zeros((Bn, N), np.float32)
    np.put_along_axis(m1, np.asarray(idx1, np.int64), 1.0, axis=1)
    m1 = m1 * (1.0 - m2)  # reference scatter order: idx2 wins collisions
    return m1.astype(ml_dtypes.bfloat16), m2.astype(ml_dtypes.bfloat16)


LAST_RESULT = None


def kernel(**inputs):
    global LAST_RESULT
    x = np.ascontiguousarray(np.asarray(inputs["x"], np.float32))
    Bn = x.shape[0]
    bl = Bn // N_CORES
    assert Bn % N_CORES == 0

    folded, SD, S1, S2f, S2q = _fold_host(inputs)
    m1, m2 = _masks_host(inputs["idx1"], inputs["idx2"], Bn)
    xpad = np.zeros((Bn, C, HP, WP), ml_dtypes.float8_e4m3fn)
    xpad[:, :, 3:31, 3:31] = x

    nc = build_bass(bl, SD, S1, S2f, S2q)

    in_maps = []
    for c in range(N_CORES):
        sl = slice(c * bl, (c + 1) * bl)
        masks = np.stack([
            m1[sl].reshape(bl, H, W), m2[sl].reshape(bl, H, W)], axis=0)
        in_maps.append(dict(
            xpad=xpad[sl],
            masks=np.ascontiguousarray(masks),
            **folded,
        ))

    trace = bool(int(os.environ.get("BASS_KERNEL_TRACE", "0")))
    res = run_bass_kernel_spmd(nc, in_maps, list(range(N_CORES)), trace=trace)
    LAST_RESULT = res
    out = np.concatenate([np.asarray(res.results[c]["out"], ml_dtypes.bfloat16)
                          for c in range(N_CORES)], axis=0).astype(np.float32)
    out += x
    return out

